# revision 1
# baseline (speedup 1.0000x reference)
"""Trainium2 Bass kernel: causal self-attention with RoPE (B=4, T=2048, D=1024, H=16, Dh=64).

Sharding: 8 cores = 4 batches x 2 head-halves. Core c handles batch c//2 and
heads (c%2)*8 .. (c%2)*8+7 (feature columns (c%2)*512 .. +512 of Wq/Wk/Wv, and
the matching rows of Wo). Each core computes a partial output [T, D]; the host
sums the two partials per batch (row-sharded Wo reduction) and stacks batches.

On-chip layout: activations are kept transposed (features on partitions):
  xT [D, T] (spilled to DRAM), qT/kT [512, T], scoresT [s, t], attn_outT [512, T].
This makes every matmul contraction land on the partition dim with zero
transposes except one PE-transpose pass over x. The softmax denominator is
fused into the AV matmul via a ones-column appended to V (M=65), and the
causal mask is applied post-exp with a single tensor_mask per diagonal group.
"""

import os
import sys

for _p in ("/opt/trn_rl_repo", "/root/.axon_site/_ro/trn_rl_repo"):
    if os.path.isdir(_p) and _p not in sys.path:
        sys.path.append(_p)

import numpy as np

import bass_rust
import concourse.bass as bass
import concourse.mybir as mybir
import concourse.tile as tile
from concourse.vector_clock import ScopedClock

F32 = mybir.dt.float32
F32R = mybir.dt.float32r
BF16 = mybir.dt.bfloat16

B, T, D, H, Dh = 4, 2048, 1024, 16, 64
FC = 512          # features per core (8 heads)
NG = 2            # head groups per core (4 heads each)
FG = FC // NG     # 256 features per group
NTC = T // 512    # 4 t-chunks
NTT = T // 128    # 16 t-tiles
ND = D // 128     # 8 d-chunks


class _TC(tile.TileContext):
    """TileContext whose tail Drain carries at most one sem wait.

    The walrus build in this container rejects a Drain with >1 sync waits
    (setupSyncWait: "Too many sync wait commands"), so spread the waits over
    a chain of Drain instructions instead.
    """

    def _drain_and_barrier(self, tick_clock, wait_clock):
        drain_inst = self.nc.sync.drain()
        wait_clock.add_sem_waits(
            drain_inst.ins, ScopedClock({None: tick_clock.global_clock})
        )
        si = drain_inst.ins.sync_info
        if si is not None and len(si.on_wait) > 1:
            waits = list(si.on_wait)
            drain_inst.ins.sync_info = bass_rust.SyncInfo(
                on_wait=waits[:1], on_update=list(si.on_update)
            )
            for w in waits[1:]:
                d2 = self.nc.sync.drain()
                d2.ins.sync_info = bass_rust.SyncInfo(on_wait=[w], on_update=[])
        self.nc.all_engine_barrier()
        popped = self.nc._tile_sem_poison_stack.pop()
        assert popped is self._sem_poison
        self.nc.clear_and_free_semaphores(list(self.sems.allocated().values()))
        self.nc.all_engine_barrier()


def _r(ap):
    return ap.bitcast(F32R)


def _split_waits(nc, max_waits=1):
    """Hoist extra sem waits onto same-engine NoOps.

    The walrus build here allows only one sync wait on several instruction
    structs (Drain, the fp32/fp32r matmul LW struct). Engine queues are
    in-order, so moving waits to a preceding NoOp on the same engine is
    semantics-preserving.
    """
    n = 0
    for fn in nc.m.functions:
        for bb in fn.blocks:
            out = []
            for inst in bb.instructions:
                si = inst.sync_info
                if si is not None and len(si.on_wait) > max_waits:
                    waits = list(si.on_wait)
                    extra, keep = waits[:-max_waits], waits[-max_waits:]
                    for i, w in enumerate(extra):
                        nop = mybir.InstNoOp(
                            name=f"{inst.name}_ws{i}", engine=inst.engine
                        )
                        nop.sync_info = bass_rust.SyncInfo(on_wait=[w], on_update=[])
                        out.append(nop)
                        n += 1
                    inst.sync_info = bass_rust.SyncInfo(
                        on_wait=keep, on_update=list(si.on_update)
                    )
                out.append(inst)
            bb.instructions = out
    return n


def _build_program():
    from contextlib import ExitStack

    nc = bass.Bass()

    x = nc.dram_tensor("x", [T, D], F32, kind="ExternalInput")
    wq = nc.dram_tensor("wq", [D, FC], F32R, kind="ExternalInput")
    wk = nc.dram_tensor("wk", [D, FC], F32R, kind="ExternalInput")
    wv = nc.dram_tensor("wv", [D, FC], F32R, kind="ExternalInput")
    wo = nc.dram_tensor("wo", [FC, D], F32R, kind="ExternalInput")
    cos2 = nc.dram_tensor("cos2", [128, T], F32, kind="ExternalInput")
    sin2 = nc.dram_tensor("sin2", [128, T], F32, kind="ExternalInput")
    ident = nc.dram_tensor("ident", [128, 128], F32, kind="ExternalInput")
    mk0 = nc.dram_tensor("mk0", [128, 1024], F32, kind="ExternalInput")
    mk256 = nc.dram_tensor("mk256", [128, 1024], F32, kind="ExternalInput")
    ones8 = nc.dram_tensor("ones8", [128, 8], F32R, kind="ExternalInput")
    ones64 = nc.dram_tensor("ones64", [1, 64], F32R, kind="ExternalInput")
    out = nc.dram_tensor("out", [T, D], F32, kind="ExternalOutput")

    with _TC(nc) as tc, ExitStack() as ctx:
        consts = ctx.enter_context(tc.tile_pool(name="consts", bufs=1))
        # PSUM: 2x [128,1024] double-bank slots + 4x [128,512] single-bank slots
        psum = ctx.enter_context(tc.tile_pool(name="psum", bufs=2, space="PSUM"))
        psums = ctx.enter_context(tc.tile_pool(name="psums", bufs=4, space="PSUM"))
        dram = ctx.enter_context(tc.tile_pool(name="dram", bufs=4, space="DRAM"))
        persist = ctx.enter_context(tc.tile_pool(name="persist", bufs=1))
        wp = ctx.enter_context(tc.tile_pool(name="wp", bufs=1))

        ident_t = consts.tile([128, 128], F32)
        nc.sync.dma_start(ident_t[:], ident[:])
        ones64_t = consts.tile([1, 64], F32R)
        nc.sync.dma_start(ones64_t[:], ones64[:])
        mk0_t = consts.tile([128, 1024], F32)
        nc.sync.dma_start(mk0_t[:], mk0[:])
        mk256_t = consts.tile([128, 1024], F32)
        nc.sync.dma_start(mk256_t[:], mk256[:])

        def load_weights(g):
            gsl = slice(g * FG, (g + 1) * FG)
            tiles = []
            for nm, wsrc in (("wq", wq), ("wk", wk)):
                w_t = wp.tile([128, ND * FG], F32R, tag=nm, name=f"{nm}_t{g}")
                nc.sync.dma_start(
                    w_t[:].rearrange("p (d f) -> p d f", d=ND),
                    wsrc[:, gsl].rearrange("(d p) f -> p d f", p=128),
                )
                tiles.append(w_t)
            return tiles

        wv_t = wp.tile([128, ND * FC], F32R, tag="wv", name="wv_t")
        nc.sync.dma_start(
            wv_t[:].rearrange("p (d f) -> p d f", d=ND),
            wv[:].rearrange("(d p) f -> p d f", p=128),
        )

        # attention outputs, persistent across both groups: 4 f-chunks [128, T]
        ao = [persist.tile([128, T], F32R, tag=f"ao{i}", name=f"ao{i}") for i in range(4)]
        # v (natural layout) with a ones column per head: 8 heads x 65 cols
        vt = [persist.tile([128, 8 * 65], F32R, tag=f"vt{i}", name=f"vt{i}") for i in range(NTT)]

        wtiles = load_weights(0)
        for i in range(NTT):
            nc.sync.dma_start(vt[i][:, 64::65], ones8[:])
        for g in range(NG):
            # per-group persistent activations (slots reused across groups)
            qt = [persist.tile([128, T], BF16, tag=f"qt{i}", name=f"qt{i}g{g}") for i in range(2)]
            kt = [persist.tile([128, T], BF16, tag=f"kt{i}", name=f"kt{i}g{g}") for i in range(2)]

            with ExitStack() as gctx:
                # ---- Phase B(g): projections + RoPE
                xtc = gctx.enter_context(tc.tile_pool(name=f"xtc{g}", bufs=3))
                rop = gctx.enter_context(tc.tile_pool(name=f"rope{g}", bufs=3))
                xld = gctx.enter_context(tc.tile_pool(name=f"xload{g}", bufs=4))

                wq_t, wk_t = wtiles

                for tcc in range(NTC):
                    csl = slice(tcc * 512, (tcc + 1) * 512)
                    cos_t = rop.tile([128, 512], F32, tag="cs", name="cos_t", bufs=2)
                    nc.sync.dma_start(cos_t[:], cos2[:, csl])
                    sin_t = rop.tile([128, 512], F32, tag="sn", name="sin_t", bufs=2)
                    nc.sync.dma_start(sin_t[:], sin2[:, csl])
                    # rotate_half sign fold: rows 0:32 / 64:96 get -sin
                    nc.scalar.mul(sin_t[0:32, :], sin_t[0:32, :], -1.0)
                    nc.scalar.mul(sin_t[64:96, :], sin_t[64:96, :], -1.0)
                    xc = xtc.tile([128, ND * 512], F32R, tag="xc")
                    # transpose x[tc] on the fly, one x-tile at a time
                    xcv = xc[:].rearrange("p (d t) -> p d t", d=ND)
                    for q in range(4):
                        t0 = (tcc * 4 + q) * 128
                        xt_ = xld.tile([128, D], F32, tag="xl")
                        nc.sync.dma_start(xt_[:], x[t0 : t0 + 128, :])
                        for dh in range(2):
                            tp = psums.tile([128, 512], F32, tag="sm", name="tp")
                            for dl in range(4):
                                d = dh * 4 + dl
                                nc.tensor.transpose(
                                    tp[:, dl * 128 : (dl + 1) * 128],
                                    xt_[:, d * 128 : (d + 1) * 128],
                                    ident_t[:],
                                )
                            nc.vector.tensor_copy(
                                xcv[:, dh * 4 : dh * 4 + 4, q * 128 : (q + 1) * 128],
                                tp[:].rearrange("p (dl t) -> p dl t", dl=4),
                            )
                    tsl = slice(tcc * 512, (tcc + 1) * 512)
                    # q/k projections (transposed outputs) + RoPE
                    for dst, w_t in ((qt, wq_t), (kt, wk_t)):
                        ps = psum.tile([128, 1024], F32, tag="pp")
                        for fp in range(2):
                            for d in range(ND):
                                nc.tensor.matmul(
                                    ps[:, fp * 512 : fp * 512 + 512],
                                    w_t[:, d * FG + fp * 128 : d * FG + (fp + 1) * 128],
                                    xc[:, d * 512 : (d + 1) * 512],
                                    start=(d == 0),
                                    stop=(d == ND - 1),
                                )
                        for fp in range(2):
                            psl = ps[:, fp * 512 : fp * 512 + 512]
                            raw = rop.tile([128, 512], F32, tag="raw")
                            nc.scalar.copy(raw[:], psl)
                            rot = rop.tile([128, 512], F32, tag="rot")
                            for hb in range(2):
                                o = hb * 64
                                nc.sync.dma_start(rot[o : o + 32, :], raw[o + 32 : o + 64, :])
                                nc.sync.dma_start(rot[o + 32 : o + 64, :], raw[o : o + 32, :])
                            dtile = dst[fp]
                            nc.vector.tensor_mul(dtile[:, tsl], psl, cos_t[:])
                            nc.vector.tensor_mul(rot[:], rot[:], sin_t[:])
                            nc.vector.tensor_add(dtile[:, tsl], dtile[:, tsl], rot[:])
                    # v projection for all 8 heads at once (group 0 only)
                    if g == 0:
                        for tb in range(4):
                            pv = psums.tile([128, 512], F32, tag="sm", name="pv")
                            for d in range(ND):
                                nc.tensor.matmul(
                                    pv[:],
                                    xc[:, d * 512 + tb * 128 : d * 512 + (tb + 1) * 128],
                                    wv_t[:, d * FC : (d + 1) * FC],
                                    start=(d == 0),
                                    stop=(d == ND - 1),
                                )
                            i = tcc * 4 + tb
                            vdst = vt[i][:, :].rearrange("p (h c) -> p h c", c=65)[
                                :, :, 0:64
                            ]
                            vsrc = pv[:].rearrange("p (h c) -> p h c", c=64)
                            nc.vector.tensor_copy(vdst, vsrc)

                if g == 0:
                    # prefetch group-1 weights while attention of group 0 runs
                    wtiles = load_weights(1)

            # ---- Phase C(g): attention. The pair's two heads are interleaved
            # so one head's QK fills the PE while the other waits on exp.
            with ExitStack() as cctx:
                ep = cctx.enter_context(tc.tile_pool(name=f"exp{g}", bufs=4))
                rp = cctx.enter_context(tc.tile_pool(name=f"rcp{g}", bufs=3))
                for tcc in (3, 2, 1, 0):
                    tsl = slice(tcc * 512, (tcc + 1) * 512)
                    ngrp = 2 * tcc + 2
                    for fp in range(2):
                        avs = []
                        for ho in range(2):
                            av_ps = psums.tile(
                                [128, 512], F32, tag="sm", name=f"av{ho}"
                            )
                            avs.append(av_ps)
                        for g2 in range(ngrp):
                            exs = []
                            for ho in range(2):
                                o = ho * 64
                                sc = psum.tile([128, 1024], F32, tag="pp", name="sc")
                                for half in range(2):
                                    si = 2 * g2 + half
                                    nc.tensor.matmul(
                                        sc[:, half * 512 : half * 512 + 512],
                                        kt[fp][o : o + 64, si * 128 : (si + 1) * 128],
                                        qt[fp][o : o + 64, tsl],
                                        start=True,
                                        stop=True,
                                    )
                                ex = ep.tile([128, 1024], F32R, tag="ex")
                                nc.scalar.activation(
                                    ex[:], sc[:], mybir.ActivationFunctionType.Exp,
                                    scale=0.125,
                                )
                                if g2 >= 2 * tcc:
                                    mt = mk0_t if g2 == 2 * tcc else mk256_t
                                    nc.vector.tensor_mul(ex[:], ex[:], mt[:])
                                exs.append(ex)
                            for ho in range(2):
                                hl = 2 * fp + ho
                                for half in range(2):
                                    si = 2 * g2 + half
                                    nc.tensor.matmul(
                                        avs[ho][0:65, :],
                                        vt[si][:, (4 * g + hl) * 65 : (4 * g + hl) * 65 + 65],
                                        exs[ho][:, half * 512 : half * 512 + 512],
                                        start=(g2 == 0 and half == 0),
                                        stop=(g2 == ngrp - 1 and half == 1),
                                    )
                        for ho in range(2):
                            o = ho * 64
                            av_sb = rp.tile([65, 512], F32, tag="avs")
                            nc.vector.tensor_copy(av_sb[:], avs[ho][0:65, :])
                            rcp = rp.tile([1, 512], F32R, tag="rc")
                            with nc.allow_low_precision(reason="f32r recip"):
                                nc.vector.reciprocal(rcp[:], av_sb[64:65, :])
                            pb = psums.tile([128, 512], F32, tag="sm", name="pb")
                            nc.tensor.matmul(
                                pb[0:64, :], ones64_t[:], rcp[:], start=True, stop=True
                            )
                            nc.vector.tensor_mul(
                                ao[2 * g + fp][o : o + 64, tsl],
                                av_sb[0:64, :],
                                pb[0:64, :],
                            )

        # ---- Phase D: output projection (row-sharded Wo partial)
        with tc.tile_pool(name="wo", bufs=1) as wop, tc.tile_pool(
            name="oev", bufs=3
        ) as oev:
            wo_t = wop.tile([128, 4 * D], F32R, tag="wo")
            nc.sync.dma_start(
                wo_t[:].rearrange("p (c o) -> p c o", c=4),
                wo[:].rearrange("(c p) o -> p c o", p=128),
            )
            for i in range(NTT):
                po = psum.tile([128, 1024], F32, tag="pp", name="po")
                for n in range(2):
                    for c in range(4):
                        nc.tensor.matmul(
                            po[:, n * 512 : n * 512 + 512],
                            ao[c][:, i * 128 : (i + 1) * 128],
                            wo_t[:, c * D + n * 512 : c * D + n * 512 + 512],
                            start=(c == 0),
                            stop=(c == 3),
                        )
                oe = oev.tile([128, 1024], F32, tag="oe")
                nc.scalar.copy(oe[:], po[:])
                nc.sync.dma_start(out[i * 128 : (i + 1) * 128, :], oe[:])

    _split_waits(nc)
    return nc

_NC_CACHE = None


def _get_nc():
    global _NC_CACHE
    if _NC_CACHE is None:
        _NC_CACHE = _build_program()
    return _NC_CACHE


def _consts():
    j = np.arange(1024)
    p = np.arange(128)
    s_rel = p[:, None] + 128 * (j[None, :] // 512)  # s offset within group
    t_rel = j[None, :] % 512
    return {
        "ident": np.eye(128, dtype=np.float32),
        "mk0": (s_rel <= t_rel).astype(np.float32),
        "mk256": (s_rel + 256 <= t_rel).astype(np.float32),
        "ones8": np.ones((128, 8), dtype=np.float32),
        "ones64": np.ones((1, 64), dtype=np.float32),
    }


def kernel(x, cos, sin, Wq, Wk, Wv, Wo):
    from concourse.bass_utils import run_bass_kernel_spmd

    x = np.asarray(x, dtype=np.float32)
    cos = np.asarray(cos, dtype=np.float32)
    sin = np.asarray(sin, dtype=np.float32)
    Wq = np.asarray(Wq, dtype=np.float32)
    Wk = np.asarray(Wk, dtype=np.float32)
    Wv = np.asarray(Wv, dtype=np.float32)
    Wo = np.asarray(Wo, dtype=np.float32)

    cos2 = np.ascontiguousarray(np.tile(cos.T, (2, 1)))  # [128, T]
    sin2 = np.ascontiguousarray(np.tile(sin.T, (2, 1)))
    consts = _consts()

    in_maps = []
    for c in range(8):
        b, hh = c // 2, c % 2
        sl = slice(hh * FC, (hh + 1) * FC)
        in_maps.append(
            {
                "x": np.ascontiguousarray(x[b]),
                "wq": np.ascontiguousarray(Wq[:, sl]),
                "wk": np.ascontiguousarray(Wk[:, sl]),
                "wv": np.ascontiguousarray(Wv[:, sl]),
                "wo": np.ascontiguousarray(Wo[sl, :]),
                "cos2": cos2,
                "sin2": sin2,
                **consts,
            }
        )

    nc = _get_nc()
    res = run_bass_kernel_spmd(nc, in_maps, core_ids=list(range(8)))
    outs = [res.results[c]["out"] for c in range(8)]
    full = np.stack([outs[2 * b] + outs[2 * b + 1] for b in range(B)])
    return full.astype(np.float32)



# revision 12
# speedup vs baseline: 1.1855x; 1.1855x over previous
"""Trainium2 Bass kernel: causal self-attention with RoPE (B=4, T=2048, D=1024, H=16, Dh=64).

Sharding: 8 cores = 4 batches x 2 head-halves. Core c handles batch c//2 and
heads (c%2)*8 .. (c%2)*8+7 (feature columns (c%2)*512 of Wq/Wk/Wv, matching
rows of Wo). Each core computes a partial output [T, D]; the host sums the two
partials per batch (row-sharded Wo reduction) and stacks batches.

v2 design (vs baseline):
  - bf16 matmul operands everywhere (fp32 PSUM accumulation).
  - x transposed once per t-chunk (PE transpose), all 8 heads projected per
    chunk; per-chunk pipeline B(proj+RoPE) -> C(attention) -> D(out-proj) so
    the ACT-bound attention overlaps the PE-bound projection of later chunks
    and the PE never idles long enough to re-throttle (HAM).
  - rotate_half via a PE permutation matmul (sign folded into the matrix)
    instead of 4 SBUF->SBUF DMA partition shifts per tile.
  - softmax denominator: ones-column in V (65th column), then
    reciprocal_approx_fast + ones-outer-product broadcast matmul (replaces
    the 3.3us single-partition exact reciprocal on the critical path).
  - QK pairs issued ho-interleaved so the two 64-row matmuls run in separate
    PE row-groups concurrently (tile_position auto-derived).
"""

import os
import sys

for _p in ("/opt/trn_rl_repo", "/root/.axon_site/_ro/trn_rl_repo"):
    if os.path.isdir(_p) and _p not in sys.path:
        sys.path.append(_p)

import numpy as np

import bass_rust
import concourse.bass as bass
import concourse.mybir as mybir
import concourse.tile as tile
from concourse.vector_clock import ScopedClock

F32 = mybir.dt.float32
F32R = mybir.dt.float32r
BF16 = mybir.dt.bfloat16

B, T, D, H, Dh = 4, 2048, 1024, 16, 64
FC = 512          # features per core (8 heads)
NP = 4            # head-pairs (fpairs) per core, 128 features each
NTC = T // 512    # 4 t-chunks
NTT = T // 128    # 16 t-tiles
ND = D // 128     # 8 d-chunks


class _TC(tile.TileContext):
    """TileContext whose tail Drain carries at most one sem wait.

    The walrus build in this container rejects a Drain with >1 sync waits
    (setupSyncWait: "Too many sync wait commands"), so spread the waits over
    a chain of Drain instructions instead.
    """

    def _drain_and_barrier(self, tick_clock, wait_clock):
        drain_inst = self.nc.sync.drain()
        wait_clock.add_sem_waits(
            drain_inst.ins, ScopedClock({None: tick_clock.global_clock})
        )
        si = drain_inst.ins.sync_info
        if si is not None and len(si.on_wait) > 1:
            waits = list(si.on_wait)
            drain_inst.ins.sync_info = bass_rust.SyncInfo(
                on_wait=waits[:1], on_update=list(si.on_update)
            )
            for w in waits[1:]:
                d2 = self.nc.sync.drain()
                d2.ins.sync_info = bass_rust.SyncInfo(on_wait=[w], on_update=[])
        self.nc.all_engine_barrier()
        popped = self.nc._tile_sem_poison_stack.pop()
        assert popped is self._sem_poison
        self.nc.clear_and_free_semaphores(list(self.sems.allocated().values()))
        self.nc.all_engine_barrier()


def _r(ap):
    return ap.bitcast(F32R)


def _split_waits(nc, max_waits=1):
    """Hoist extra sem waits onto same-engine NoOps.

    The walrus build here allows only one sync wait on several instruction
    structs (Drain, the fp32/fp32r matmul LW struct). Engine queues are
    in-order, so moving waits to a preceding NoOp on the same engine is
    semantics-preserving.
    """
    n = 0
    for fn in nc.m.functions:
        for bb in fn.blocks:
            out = []
            for inst in bb.instructions:
                si = inst.sync_info
                if si is not None and len(si.on_wait) > max_waits:
                    waits = list(si.on_wait)
                    extra, keep = waits[:-max_waits], waits[-max_waits:]
                    for i, w in enumerate(extra):
                        nop = mybir.InstNoOp(
                            name=f"{inst.name}_ws{i}", engine=inst.engine
                        )
                        nop.sync_info = bass_rust.SyncInfo(on_wait=[w], on_update=[])
                        out.append(nop)
                        n += 1
                    inst.sync_info = bass_rust.SyncInfo(
                        on_wait=keep, on_update=list(si.on_update)
                    )
                out.append(inst)
            bb.instructions = out
    return n


def _build_program():
    from contextlib import ExitStack

    nc = bass.Bass()

    x = nc.dram_tensor("x", [T, D], F32, kind="ExternalInput")
    wq = nc.dram_tensor("wq", [D, FC], BF16, kind="ExternalInput")
    wk = nc.dram_tensor("wk", [D, FC], BF16, kind="ExternalInput")
    wv = nc.dram_tensor("wv", [D, FC], BF16, kind="ExternalInput")
    wo = nc.dram_tensor("wo", [FC, D], BF16, kind="ExternalInput")
    cos2 = nc.dram_tensor("cos2", [128, T], F32, kind="ExternalInput")
    sin2 = nc.dram_tensor("sin2", [128, T], F32, kind="ExternalInput")
    ident = nc.dram_tensor("ident", [128, 128], F32, kind="ExternalInput")
    spt = nc.dram_tensor("spt", [128, 128], BF16, kind="ExternalInput")
    mk0 = nc.dram_tensor("mk0", [128, 1024], BF16, kind="ExternalInput")
    mk256 = nc.dram_tensor("mk256", [128, 1024], BF16, kind="ExternalInput")
    ones8 = nc.dram_tensor("ones8", [128, 8], BF16, kind="ExternalInput")
    ones64 = nc.dram_tensor("ones64", [1, 64], F32R, kind="ExternalInput")
    out = nc.dram_tensor("out", [T, D], F32, kind="ExternalOutput")

    with _TC(nc) as tc, ExitStack() as ctx:
        consts = ctx.enter_context(tc.tile_pool(name="consts", bufs=1))
        # PSUM: 2x [128,1024] (4 banks, scores) + 2x [128,512] (avs)
        #     + 2x [128,512] (transposes/proj/rot/v/out-proj)
        psc = ctx.enter_context(tc.tile_pool(name="psc", bufs=2, space="PSUM"))
        pav = ctx.enter_context(tc.tile_pool(name="pav", bufs=2, space="PSUM"))
        psb = ctx.enter_context(tc.tile_pool(name="psb", bufs=2, space="PSUM"))
        persist = ctx.enter_context(tc.tile_pool(name="persist", bufs=1))
        wp = ctx.enter_context(tc.tile_pool(name="wp", bufs=1))
        xld = ctx.enter_context(tc.tile_pool(name="xload", bufs=3))
        xtc = ctx.enter_context(tc.tile_pool(name="xtc", bufs=2))
        rop = ctx.enter_context(tc.tile_pool(name="rope", bufs=3))
        ep = ctx.enter_context(tc.tile_pool(name="exp", bufs=4))
        rp = ctx.enter_context(tc.tile_pool(name="rcp", bufs=3))
        oev = ctx.enter_context(tc.tile_pool(name="oev", bufs=3))

        ident_t = consts.tile([128, 128], F32)
        nc.sync.dma_start(ident_t[:], ident[:])
        spt_t = consts.tile([128, 128], BF16)
        nc.sync.dma_start(spt_t[:], spt[:])
        ones64_t = consts.tile([1, 64], F32R)
        nc.sync.dma_start(ones64_t[:], ones64[:])
        mk0_t = consts.tile([128, 1024], BF16)
        nc.sync.dma_start(mk0_t[:], mk0[:])
        mk256_t = consts.tile([128, 1024], BF16)
        nc.sync.dma_start(mk256_t[:], mk256[:])
        cos_t = consts.tile([128, T], F32)
        nc.sync.dma_start(cos_t[:], cos2[:])
        sin_t = consts.tile([128, T], F32)
        nc.sync.dma_start(sin_t[:], sin2[:])

        # weights
        wq_t = wp.tile([128, ND * FC], BF16, tag="wq", name="wq_t")
        nc.sync.dma_start(
            wq_t[:].rearrange("p (d f) -> p d f", d=ND),
            wq[:].rearrange("(d p) f -> p d f", p=128),
        )
        wk_t = wp.tile([128, ND * FC], BF16, tag="wk", name="wk_t")
        nc.sync.dma_start(
            wk_t[:].rearrange("p (d f) -> p d f", d=ND),
            wk[:].rearrange("(d p) f -> p d f", p=128),
        )
        wv_t = wp.tile([128, ND * FC], BF16, tag="wv", name="wv_t")
        nc.sync.dma_start(
            wv_t[:].rearrange("p (d f) -> p d f", d=ND),
            wv[:].rearrange("(d p) f -> p d f", p=128),
        )
        wo_t = wp.tile([128, 4 * D], BF16, tag="wo", name="wo_t")
        nc.sync.dma_start(
            wo_t[:].rearrange("p (c o) -> p c o", c=4),
            wo[:].rearrange("(c p) o -> p c o", p=128),
        )

        # persistent activations
        # q/k transposed per fpair: [128 feat, T] bf16
        qt = [persist.tile([128, T], BF16, tag=f"qt{i}", name=f"qt{i}") for i in range(NP)]
        kt = [persist.tile([128, T], BF16, tag=f"kt{i}", name=f"kt{i}") for i in range(NP)]
        # v natural layout per s-tile with a ones column per head: 8 x 65
        vt = [persist.tile([128, 8 * 65], BF16, tag=f"vt{i}", name=f"vt{i}") for i in range(NTT)]
        # attention output (normalized) per fpair: [128 feat, T] bf16
        ao = [persist.tile([128, T], BF16, tag=f"ao{i}", name=f"ao{i}") for i in range(NP)]

        for i in range(NTT):
            nc.sync.dma_start(vt[i][:, 64::65], ones8[:])

        for tcc in range(NTC):
            tsl = slice(tcc * 512, (tcc + 1) * 512)

            # ---- B(tcc): transpose x chunk, project q/k/v, RoPE
            xc = xtc.tile([128, ND * 512], BF16, tag="xc")
            xcv = xc[:].rearrange("p (d t) -> p d t", d=ND)
            for q in range(4):
                t0 = (tcc * 4 + q) * 128
                xt_ = xld.tile([128, D], F32, tag="xl")
                nc.sync.dma_start(xt_[:], x[t0 : t0 + 128, :])
                for dh in range(2):
                    tp = psb.tile([128, 512], F32, tag="sm", name="tp")
                    for dl in range(4):
                        d = dh * 4 + dl
                        nc.tensor.transpose(
                            tp[:, dl * 128 : (dl + 1) * 128],
                            xt_[:, d * 128 : (d + 1) * 128],
                            ident_t[:],
                        )
                    nc.vector.tensor_copy(
                        xcv[:, dh * 4 : dh * 4 + 4, q * 128 : (q + 1) * 128],
                        tp[:].rearrange("p (dl t) -> p dl t", dl=4),
                    )

            # q/k projections + RoPE, per fpair
            for dst, w_t in ((qt, wq_t), (kt, wk_t)):
                for fp in range(NP):
                    ps = psb.tile([128, 512], F32, tag="sm", name="ps")
                    for d in range(ND):
                        nc.tensor.matmul(
                            ps[:],
                            w_t[:, d * FC + fp * 128 : d * FC + (fp + 1) * 128],
                            xc[:, d * 512 : (d + 1) * 512],
                            start=(d == 0),
                            stop=(d == ND - 1),
                        )
                    raw = rop.tile([128, 512], BF16, tag="raw")
                    nc.scalar.copy(raw[:], ps[:])
                    rot = psb.tile([128, 512], F32, tag="sm", name="rot")
                    nc.tensor.matmul(
                        rot[:], spt_t[:], raw[:], start=True, stop=True
                    )
                    dtile = dst[fp]
                    tmp = rop.tile([128, 512], BF16, tag="tmp")
                    nc.vector.tensor_mul(dtile[:, tsl], ps[:], cos_t[:, tsl])
                    nc.vector.tensor_mul(tmp[:], rot[:], sin_t[:, tsl])
                    nc.vector.tensor_add(dtile[:, tsl], dtile[:, tsl], tmp[:])

            # v projection (all 8 heads per t-tile)
            for tb in range(4):
                pv = psb.tile([128, 512], F32, tag="sm", name="pv")
                for d in range(ND):
                    nc.tensor.matmul(
                        pv[:],
                        xc[:, d * 512 + tb * 128 : d * 512 + (tb + 1) * 128],
                        wv_t[:, d * FC : (d + 1) * FC],
                        start=(d == 0),
                        stop=(d == ND - 1),
                    )
                i = tcc * 4 + tb
                vdst = vt[i][:, :].rearrange("p (h c) -> p h c", c=65)[:, :, 0:64]
                vsrc = pv[:].rearrange("p (h c) -> p h c", c=64)
                nc.vector.tensor_copy(vdst, vsrc)

            # ---- C(tcc): attention for this t-chunk over s-tiles 0..4*tcc+3
            ngrp = 2 * tcc + 2
            for fp in range(NP):
                avs = [
                    pav.tile([128, 512], F32, tag="av", name=f"av{ho}")
                    for ho in range(2)
                ]
                for g2 in range(ngrp):
                    scs = []
                    for ho in range(2):
                        scs.append(psc.tile([128, 1024], F32, tag="sc", name="sc"))
                    # QK interleaved so the two 64-row matmuls overlap in
                    # different PE row groups
                    for half in range(2):
                        si = 2 * g2 + half
                        for ho in range(2):
                            o = ho * 64
                            nc.tensor.matmul(
                                scs[ho][:, half * 512 : half * 512 + 512],
                                kt[fp][o : o + 64, si * 128 : (si + 1) * 128],
                                qt[fp][o : o + 64, tsl],
                                start=True,
                                stop=True,
                            )
                    exs = []
                    for ho in range(2):
                        ex = ep.tile([128, 1024], BF16, tag="ex")
                        nc.scalar.activation(
                            ex[:], scs[ho][:], mybir.ActivationFunctionType.Exp,
                            scale=0.125,
                        )
                        if g2 >= 2 * tcc:
                            mt = mk0_t if g2 == 2 * tcc else mk256_t
                            nc.vector.tensor_mul(ex[:], ex[:], mt[:])
                        exs.append(ex)
                    for ho in range(2):
                        h = 2 * fp + ho
                        for half in range(2):
                            si = 2 * g2 + half
                            nc.tensor.matmul(
                                avs[ho][0:65, :],
                                vt[si][:, h * 65 : h * 65 + 65],
                                exs[ho][:, half * 512 : half * 512 + 512],
                                start=(g2 == 0 and half == 0),
                                stop=(g2 == ngrp - 1 and half == 1),
                            )
                # softmax normalize: 1/denominator broadcast via ones outer
                for ho in range(2):
                    o = ho * 64
                    av_sb = rp.tile([65, 512], F32, tag="avs")
                    nc.vector.tensor_copy(av_sb[:], avs[ho][0:65, :])
                    # reciprocal as exp(-ln(x)) on the scalar engine: the DVE
                    # exact reciprocal costs 3.3us on the critical path and
                    # custom-DVE approx ops don't compile in this walrus build
                    lnt = rp.tile([1, 512], F32, tag="ln")
                    nc.scalar.activation(
                        lnt[:], av_sb[64:65, :], mybir.ActivationFunctionType.Ln
                    )
                    rcp = rp.tile([1, 512], F32R, tag="rc")
                    nc.scalar.activation(
                        rcp[:], lnt[:], mybir.ActivationFunctionType.Exp, scale=-1.0
                    )
                    pb = psb.tile([128, 512], F32, tag="sm", name="pb")
                    nc.tensor.matmul(
                        pb[0:64, :], ones64_t[:], rcp[:], start=True, stop=True
                    )
                    nc.vector.tensor_mul(
                        ao[fp][o : o + 64, tsl], av_sb[0:64, :], pb[0:64, :]
                    )

            # ---- D(tcc): output projection for this t-chunk
            for tb in range(4):
                i = tcc * 4 + tb
                for n in range(2):
                    po = psb.tile([128, 512], F32, tag="sm", name="po")
                    for c in range(4):
                        nc.tensor.matmul(
                            po[:],
                            ao[c][:, i * 128 : (i + 1) * 128],
                            wo_t[:, c * D + n * 512 : c * D + n * 512 + 512],
                            start=(c == 0),
                            stop=(c == 3),
                        )
                    oe = oev.tile([128, 512], F32, tag="oe")
                    nc.scalar.copy(oe[:], po[:])
                    nc.sync.dma_start(
                        out[i * 128 : (i + 1) * 128, n * 512 : n * 512 + 512], oe[:]
                    )

    _split_waits(nc)
    return nc


_NC_CACHE = None


def _get_nc():
    global _NC_CACHE
    if _NC_CACHE is None:
        _NC_CACHE = _build_program()
    return _NC_CACHE


def _consts():
    import ml_dtypes

    bf16 = ml_dtypes.bfloat16
    j = np.arange(1024)
    p = np.arange(128)
    s_rel = p[:, None] + 128 * (j[None, :] // 512)  # s offset within group
    t_rel = j[None, :] % 512
    # rotate_half permutation-with-sign, transposed for matmul lhsT:
    # rot = SP @ raw, SP[f, f+32] = -1 (f%64 < 32), SP[f, f-32] = +1.
    sp = np.zeros((128, 128), dtype=np.float32)
    for b in (0, 64):
        for i in range(32):
            sp[b + i, b + i + 32] = -1.0
            sp[b + i + 32, b + i] = 1.0
    return {
        "ident": np.eye(128, dtype=np.float32),
        "spt": np.ascontiguousarray(sp.T).astype(bf16),
        "mk0": (s_rel <= t_rel).astype(bf16),
        "mk256": (s_rel + 256 <= t_rel).astype(bf16),
        "ones8": np.ones((128, 8), dtype=bf16),
        "ones64": np.ones((1, 64), dtype=np.float32),
    }


def _in_maps(x, cos, sin, Wq, Wk, Wv, Wo):
    import ml_dtypes

    bf16 = ml_dtypes.bfloat16
    x = np.asarray(x, dtype=np.float32)
    cos = np.asarray(cos, dtype=np.float32)
    sin = np.asarray(sin, dtype=np.float32)
    Wq = np.asarray(Wq, dtype=np.float32)
    Wk = np.asarray(Wk, dtype=np.float32)
    Wv = np.asarray(Wv, dtype=np.float32)
    Wo = np.asarray(Wo, dtype=np.float32)

    cos2 = np.ascontiguousarray(np.tile(cos.T, (2, 1)))  # [128, T]
    sin2 = np.ascontiguousarray(np.tile(sin.T, (2, 1)))
    consts = _consts()

    in_maps = []
    for c in range(8):
        b, hh = c // 2, c % 2
        sl = slice(hh * FC, (hh + 1) * FC)
        in_maps.append(
            {
                "x": np.ascontiguousarray(x[b]),
                "wq": np.ascontiguousarray(Wq[:, sl]).astype(bf16),
                "wk": np.ascontiguousarray(Wk[:, sl]).astype(bf16),
                "wv": np.ascontiguousarray(Wv[:, sl]).astype(bf16),
                "wo": np.ascontiguousarray(Wo[sl, :]).astype(bf16),
                "cos2": cos2,
                "sin2": sin2,
                **consts,
            }
        )
    return in_maps


def _gather(results):
    outs = [results[c]["out"] for c in range(8)]
    return np.stack([outs[2 * b] + outs[2 * b + 1] for b in range(B)]).astype(
        np.float32
    )


def kernel(x, cos, sin, Wq, Wk, Wv, Wo):
    from concourse.bass_utils import run_bass_kernel_spmd

    in_maps = _in_maps(x, cos, sin, Wq, Wk, Wv, Wo)
    nc = _get_nc()
    res = run_bass_kernel_spmd(nc, in_maps, core_ids=list(range(8)))
    return _gather(res.results)


# revision 14
# speedup vs baseline: 1.3696x; 1.1553x over previous
"""Trainium2 Bass kernel: causal self-attention with RoPE (B=4, T=2048, D=1024, H=16, Dh=64).

Sharding: 8 cores = 4 batches x 2 head-halves. Core c handles batch c//2 and
heads (c%2)*8 .. (c%2)*8+7 (feature columns (c%2)*512 of Wq/Wk/Wv, matching
rows of Wo). Each core computes a partial output [T, D]; the host sums the two
partials per batch (row-sharded Wo reduction) and stacks batches.

v3 design:
  - bf16 matmul operands everywhere (fp32 PSUM accumulation).
  - software-pipelined phases: projection work for t-chunk tcc+1 is emitted
    interleaved into the attention phase of chunk tcc (at head-pair
    boundaries), and the output-projection phases are deferred into the last
    attention phase, so the PE always has dense independent matmul work and
    the HAM clock gate stays at 8/8.
  - per-chunk q/k/ao tiles (no false cross-phase dependencies), PSUM pools
    split so projection never waits on attention slots.
  - causal diagonal trimmed at 128-column granularity in QK, exp, AV; the
    mask is one shared 128x128 lower-triangular multiply on the (otherwise
    idle) GpSimd engine.
  - rotate_half via a PE permutation matmul (sign folded into the matrix).
  - softmax denominator: ones-column in V, reciprocal as exp(-ln(x)) on the
    scalar engine, broadcast across partitions with a ones-outer-product
    matmul, one multiply on DVE.
"""

import os
import sys

for _p in ("/opt/trn_rl_repo", "/root/.axon_site/_ro/trn_rl_repo"):
    if os.path.isdir(_p) and _p not in sys.path:
        sys.path.append(_p)

import numpy as np

import bass_rust
import concourse.bass as bass
import concourse.mybir as mybir
import concourse.tile as tile
from concourse.vector_clock import ScopedClock

F32 = mybir.dt.float32
F32R = mybir.dt.float32r
BF16 = mybir.dt.bfloat16

B, T, D, H, Dh = 4, 2048, 1024, 16, 64
FC = 512          # features per core (8 heads)
NP = 4            # head-pairs (fpairs) per core, 128 features each
NTC = T // 512    # 4 t-chunks
NTT = T // 128    # 16 t-tiles
ND = D // 128     # 8 d-chunks


class _TC(tile.TileContext):
    """TileContext whose tail Drain carries at most one sem wait.

    The walrus build in this container rejects a Drain with >1 sync waits
    (setupSyncWait: "Too many sync wait commands"), so spread the waits over
    a chain of Drain instructions instead.
    """

    def _drain_and_barrier(self, tick_clock, wait_clock):
        drain_inst = self.nc.sync.drain()
        wait_clock.add_sem_waits(
            drain_inst.ins, ScopedClock({None: tick_clock.global_clock})
        )
        si = drain_inst.ins.sync_info
        if si is not None and len(si.on_wait) > 1:
            waits = list(si.on_wait)
            drain_inst.ins.sync_info = bass_rust.SyncInfo(
                on_wait=waits[:1], on_update=list(si.on_update)
            )
            for w in waits[1:]:
                d2 = self.nc.sync.drain()
                d2.ins.sync_info = bass_rust.SyncInfo(on_wait=[w], on_update=[])
        self.nc.all_engine_barrier()
        popped = self.nc._tile_sem_poison_stack.pop()
        assert popped is self._sem_poison
        self.nc.clear_and_free_semaphores(list(self.sems.allocated().values()))
        self.nc.all_engine_barrier()


def _r(ap):
    return ap.bitcast(F32R)


def _split_waits(nc, max_waits=1):
    """Hoist extra sem waits onto same-engine NoOps.

    The walrus build here allows only one sync wait on several instruction
    structs (Drain, the fp32/fp32r matmul LW struct, the gpsimd TT struct).
    Engine queues are in-order, so moving waits to a preceding NoOp on the
    same engine is semantics-preserving.
    """
    n = 0
    for fn in nc.m.functions:
        for bb in fn.blocks:
            out = []
            for inst in bb.instructions:
                si = inst.sync_info
                if si is not None and len(si.on_wait) > max_waits:
                    waits = list(si.on_wait)
                    extra, keep = waits[:-max_waits], waits[-max_waits:]
                    for i, w in enumerate(extra):
                        nop = mybir.InstNoOp(
                            name=f"{inst.name}_ws{i}", engine=inst.engine
                        )
                        nop.sync_info = bass_rust.SyncInfo(on_wait=[w], on_update=[])
                        out.append(nop)
                        n += 1
                    inst.sync_info = bass_rust.SyncInfo(
                        on_wait=keep, on_update=list(si.on_update)
                    )
                out.append(inst)
            bb.instructions = out
    return n


def _build_program():
    from contextlib import ExitStack

    nc = bass.Bass()

    x = nc.dram_tensor("x", [T, D], F32, kind="ExternalInput")
    wq = nc.dram_tensor("wq", [D, FC], BF16, kind="ExternalInput")
    wk = nc.dram_tensor("wk", [D, FC], BF16, kind="ExternalInput")
    wv = nc.dram_tensor("wv", [D, FC], BF16, kind="ExternalInput")
    wo = nc.dram_tensor("wo", [FC, D], BF16, kind="ExternalInput")
    cos2 = nc.dram_tensor("cos2", [128, T], F32, kind="ExternalInput")
    sin2 = nc.dram_tensor("sin2", [128, T], F32, kind="ExternalInput")
    ident = nc.dram_tensor("ident", [128, 128], F32, kind="ExternalInput")
    spt = nc.dram_tensor("spt", [128, 128], BF16, kind="ExternalInput")
    tri = nc.dram_tensor("tri", [128, 128], BF16, kind="ExternalInput")
    ones8 = nc.dram_tensor("ones8", [128, 8], BF16, kind="ExternalInput")
    ones64 = nc.dram_tensor("ones64", [1, 64], F32R, kind="ExternalInput")
    out = nc.dram_tensor("out", [T, D], F32, kind="ExternalOutput")

    with _TC(nc) as tc, ExitStack() as ctx:
        consts = ctx.enter_context(tc.tile_pool(name="consts", bufs=1))
        # PSUM (8 banks): 2x[128,1024] scores/broadcast/out-proj
        #               + 2x[128,512] attention accumulators
        #               + 2x[128,512] projection-only (transpose/proj/rot/v)
        psc = ctx.enter_context(tc.tile_pool(name="psc", bufs=2, space="PSUM"))
        pav = ctx.enter_context(tc.tile_pool(name="pav", bufs=2, space="PSUM"))
        psb = ctx.enter_context(tc.tile_pool(name="psb", bufs=2, space="PSUM"))
        persist = ctx.enter_context(tc.tile_pool(name="persist", bufs=1))
        wp = ctx.enter_context(tc.tile_pool(name="wp", bufs=1))
        xld = ctx.enter_context(tc.tile_pool(name="xload", bufs=4))
        xtc = ctx.enter_context(tc.tile_pool(name="xtc", bufs=2))
        rop = ctx.enter_context(tc.tile_pool(name="rope", bufs=3))
        ep = ctx.enter_context(tc.tile_pool(name="exp", bufs=4))
        rp = ctx.enter_context(tc.tile_pool(name="rcp", bufs=3))
        oev = ctx.enter_context(tc.tile_pool(name="oev", bufs=3))

        ident_t = consts.tile([128, 128], F32)
        nc.sync.dma_start(ident_t[:], ident[:])
        spt_t = consts.tile([128, 128], BF16)
        nc.sync.dma_start(spt_t[:], spt[:])
        tri_t = consts.tile([128, 128], BF16)
        nc.sync.dma_start(tri_t[:], tri[:])
        ones64_t = consts.tile([1, 64], F32R)
        nc.sync.dma_start(ones64_t[:], ones64[:])
        cos_t = consts.tile([128, T], F32)
        nc.sync.dma_start(cos_t[:], cos2[:])
        sin_t = consts.tile([128, T], F32)
        nc.sync.dma_start(sin_t[:], sin2[:])

        wq_t = wp.tile([128, ND * FC], BF16, tag="wq", name="wq_t")
        nc.sync.dma_start(
            wq_t[:].rearrange("p (d f) -> p d f", d=ND),
            wq[:].rearrange("(d p) f -> p d f", p=128),
        )
        wk_t = wp.tile([128, ND * FC], BF16, tag="wk", name="wk_t")
        nc.sync.dma_start(
            wk_t[:].rearrange("p (d f) -> p d f", d=ND),
            wk[:].rearrange("(d p) f -> p d f", p=128),
        )
        wv_t = wp.tile([128, ND * FC], BF16, tag="wv", name="wv_t")
        nc.sync.dma_start(
            wv_t[:].rearrange("p (d f) -> p d f", d=ND),
            wv[:].rearrange("(d p) f -> p d f", p=128),
        )
        wo_t = wp.tile([128, 4 * D], BF16, tag="wo", name="wo_t")
        nc.sync.dma_start(
            wo_t[:].rearrange("p (c o) -> p c o", c=4),
            wo[:].rearrange("(c p) o -> p c o", p=128),
        )

        # persistent activations, per t-chunk so cross-phase accesses never
        # alias (tile-granular dependency tracking would serialize them)
        qt = [
            [
                persist.tile(
                    [128, 512], BF16, tag=f"qt{t}_{i}", name=f"qt{t}_{i}"
                )
                for i in range(NP)
            ]
            for t in range(NTC)
        ]
        kt = [
            [
                persist.tile(
                    [128, 512], BF16, tag=f"kt{t}_{i}", name=f"kt{t}_{i}"
                )
                for i in range(NP)
            ]
            for t in range(NTC)
        ]
        ao = [
            [
                persist.tile(
                    [128, 512], BF16, tag=f"ao{t}_{i}", name=f"ao{t}_{i}"
                )
                for i in range(NP)
            ]
            for t in range(NTC)
        ]
        vt = [
            persist.tile([128, 8 * 65], BF16, tag=f"vt{i}", name=f"vt{i}")
            for i in range(NTT)
        ]
        for i in range(NTT):
            nc.sync.dma_start(vt[i][:, 64::65], ones8[:])

        xcs = {}

        def b_load(tcc):
            """x chunk load + PE transpose -> xc[tcc] (bf16, d-major)."""
            xc = xtc.tile([128, ND * 512], BF16, tag="xc", name=f"xc{tcc}")
            xcs[tcc] = xc
            xcv = xc[:].rearrange("p (d t) -> p d t", d=ND)
            for q in range(4):
                t0 = (tcc * 4 + q) * 128
                xt_ = xld.tile([128, D], F32, tag="xl")
                nc.sync.dma_start(xt_[:], x[t0 : t0 + 128, :])
                for dh in range(2):
                    tp = psb.tile([128, 512], F32, tag="sm", name="tp")
                    for dl in range(4):
                        d = dh * 4 + dl
                        nc.tensor.transpose(
                            tp[:, dl * 128 : (dl + 1) * 128],
                            xt_[:, d * 128 : (d + 1) * 128],
                            ident_t[:],
                        )
                    nc.vector.tensor_copy(
                        xcv[:, dh * 4 : dh * 4 + 4, q * 128 : (q + 1) * 128],
                        tp[:].rearrange("p (dl t) -> p dl t", dl=4),
                    )

        def b_proj(tcc, which):
            """q or k projection + RoPE for all 4 fpairs of chunk tcc."""
            tsl = slice(tcc * 512, (tcc + 1) * 512)
            w_t = wq_t if which == "q" else wk_t
            dst = qt[tcc] if which == "q" else kt[tcc]
            xc = xcs[tcc]
            for fp in range(NP):
                ps = psb.tile([128, 512], F32, tag="sm", name="ps")
                for d in range(ND):
                    nc.tensor.matmul(
                        ps[:],
                        w_t[:, d * FC + fp * 128 : d * FC + (fp + 1) * 128],
                        xc[:, d * 512 : (d + 1) * 512],
                        start=(d == 0),
                        stop=(d == ND - 1),
                    )
                raw = rop.tile([128, 512], BF16, tag="raw")
                nc.vector.tensor_copy(raw[:], ps[:])
                rot = psb.tile([128, 512], F32, tag="sm", name="rot")
                nc.tensor.matmul(rot[:], spt_t[:], raw[:], start=True, stop=True)
                dtile = dst[fp]
                tmp = rop.tile([128, 512], BF16, tag="tmp")
                nc.vector.tensor_mul(dtile[:], ps[:], cos_t[:, tsl])
                nc.vector.tensor_mul(tmp[:], rot[:], sin_t[:, tsl])
                nc.vector.tensor_add(dtile[:], dtile[:], tmp[:])

        def b_projv(tcc):
            """v projection (all 8 heads) for chunk tcc."""
            xc = xcs[tcc]
            for tb in range(4):
                pv = psb.tile([128, 512], F32, tag="sm", name="pv")
                for d in range(ND):
                    nc.tensor.matmul(
                        pv[:],
                        xc[:, d * 512 + tb * 128 : d * 512 + (tb + 1) * 128],
                        wv_t[:, d * FC : (d + 1) * FC],
                        start=(d == 0),
                        stop=(d == ND - 1),
                    )
                i = tcc * 4 + tb
                vdst = vt[i][:, :].rearrange("p (h c) -> p h c", c=65)[:, :, 0:64]
                vsrc = pv[:].rearrange("p (h c) -> p h c", c=64)
                nc.vector.tensor_copy(vdst, vsrc)

        def qk_mm(sc, si, fp, ho, ccol, n):
            """scores for s-tile si, t columns [ccol, ccol+n) of the chunk."""
            o = ho * 64
            tc2 = si // 4
            nc.tensor.matmul(
                sc,
                kt[tc2][fp][o : o + 64, (si % 4) * 128 : (si % 4) * 128 + 128],
                qt[tcc_cur[0]][fp][o : o + 64, ccol : ccol + n],
                start=True,
                stop=True,
            )

        tcc_cur = [0]

        def c_fpair(tcc, fp):
            tcc_cur[0] = tcc
            ngrp = 2 * tcc + 2
            avs = [
                pav.tile([128, 512], F32, tag="av", name=f"av{ho}") for ho in range(2)
            ]
            for g2 in range(ngrp):
                diag_lo = g2 == 2 * tcc
                diag_hi = g2 == 2 * tcc + 1
                scs = [
                    psc.tile([128, 1024], F32, tag="sc", name="sc") for _ in range(2)
                ]
                # QK, ho-interleaved so the two 64-row matmuls run in
                # different PE row groups concurrently
                if diag_lo:
                    for ho in range(2):
                        qk_mm(scs[ho][:, 0:512], 4 * tcc, fp, ho, 0, 512)
                    for ho in range(2):
                        qk_mm(scs[ho][:, 640:1024], 4 * tcc + 1, fp, ho, 128, 384)
                elif diag_hi:
                    for ho in range(2):
                        qk_mm(scs[ho][:, 256:512], 4 * tcc + 2, fp, ho, 256, 256)
                    for ho in range(2):
                        qk_mm(scs[ho][:, 896:1024], 4 * tcc + 3, fp, ho, 384, 128)
                else:
                    for half in range(2):
                        si = 2 * g2 + half
                        for ho in range(2):
                            qk_mm(
                                scs[ho][:, half * 512 : half * 512 + 512],
                                si, fp, ho, 0, 512,
                            )
                exs = []
                for ho in range(2):
                    ex = ep.tile([128, 1024], BF16, tag="ex")
                    if diag_hi:
                        nc.scalar.activation(
                            ex[:, 256:512], scs[ho][:, 256:512],
                            mybir.ActivationFunctionType.Exp, scale=0.125,
                        )
                        nc.scalar.activation(
                            ex[:, 896:1024], scs[ho][:, 896:1024],
                            mybir.ActivationFunctionType.Exp, scale=0.125,
                        )
                        nc.gpsimd.tensor_mul(
                            ex[:, 256:384], ex[:, 256:384], tri_t[:]
                        )
                        nc.gpsimd.tensor_mul(
                            ex[:, 896:1024], ex[:, 896:1024], tri_t[:]
                        )
                    else:
                        nc.scalar.activation(
                            ex[:], scs[ho][:],
                            mybir.ActivationFunctionType.Exp, scale=0.125,
                        )
                        if diag_lo:
                            nc.gpsimd.tensor_mul(ex[:, 0:128], ex[:, 0:128], tri_t[:])
                            nc.gpsimd.tensor_mul(
                                ex[:, 640:768], ex[:, 640:768], tri_t[:]
                            )
                    exs.append(ex)
                for ho in range(2):
                    h = 2 * fp + ho
                    vcol = slice(h * 65, h * 65 + 65)
                    first = g2 == 0
                    last = g2 == ngrp - 1
                    if diag_lo:
                        nc.tensor.matmul(
                            avs[ho][0:65, 0:512], vt[4 * tcc][:, vcol],
                            exs[ho][:, 0:512], start=first, stop=False,
                        )
                        nc.tensor.matmul(
                            avs[ho][0:65, 128:512], vt[4 * tcc + 1][:, vcol],
                            exs[ho][:, 640:1024], start=False, stop=last and False,
                        )
                    elif diag_hi:
                        nc.tensor.matmul(
                            avs[ho][0:65, 256:512], vt[4 * tcc + 2][:, vcol],
                            exs[ho][:, 256:512], start=False, stop=False,
                        )
                        nc.tensor.matmul(
                            avs[ho][0:65, 384:512], vt[4 * tcc + 3][:, vcol],
                            exs[ho][:, 896:1024], start=False, stop=last,
                        )
                    else:
                        for half in range(2):
                            si = 2 * g2 + half
                            nc.tensor.matmul(
                                avs[ho][0:65, 0:512], vt[si][:, vcol],
                                exs[ho][:, half * 512 : half * 512 + 512],
                                start=first and half == 0, stop=False,
                            )
            # softmax normalize
            for ho in range(2):
                o = ho * 64
                av_sb = rp.tile([65, 512], F32, tag="avs")
                nc.vector.tensor_copy(av_sb[:], avs[ho][0:65, :])
                # reciprocal as exp(-ln(x)) on the scalar engine (the DVE
                # exact reciprocal costs 3.3us; custom-DVE ops don't compile
                # in this walrus build)
                lnt = rp.tile([1, 512], F32, tag="ln")
                nc.scalar.activation(
                    lnt[:], av_sb[64:65, :], mybir.ActivationFunctionType.Ln
                )
                rcp = rp.tile([1, 512], F32R, tag="rc")
                nc.scalar.activation(
                    rcp[:], lnt[:], mybir.ActivationFunctionType.Exp, scale=-1.0
                )
                pb = psc.tile([128, 1024], F32, tag="sc", name="pb")
                nc.tensor.matmul(
                    pb[0:64, 0:512], ones64_t[:], rcp[:], start=True, stop=True
                )
                nc.vector.tensor_mul(
                    ao[tcc][fp][o : o + 64, :], av_sb[0:64, :], pb[0:64, 0:512]
                )

        def d_phase(tcc):
            for tb in range(4):
                i = tcc * 4 + tb
                po = psc.tile([128, 1024], F32, tag="sc", name="po")
                for n in range(2):
                    for c in range(4):
                        nc.tensor.matmul(
                            po[:, n * 512 : n * 512 + 512],
                            ao[tcc][c][:, tb * 128 : (tb + 1) * 128],
                            wo_t[:, c * D + n * 512 : c * D + n * 512 + 512],
                            start=(c == 0),
                            stop=(c == 3),
                        )
                oe = oev.tile([128, 1024], F32, tag="oe")
                nc.vector.tensor_copy(oe[:], po[:])
                nc.sync.dma_start(out[i * 128 : (i + 1) * 128, :], oe[:])

        b_load(0)
        b_proj(0, "q")
        b_proj(0, "k")
        b_projv(0)
        bp = [b_load, lambda t: b_proj(t, "q"), lambda t: b_proj(t, "k"), b_projv]
        for tcc in range(NTC):
            for fp in range(NP):
                c_fpair(tcc, fp)
                if tcc < NTC - 1:
                    bp[fp](tcc + 1)
                else:
                    d_phase(fp)

    _split_waits(nc)
    return nc


_NC_CACHE = None


def _get_nc():
    global _NC_CACHE
    if _NC_CACHE is None:
        _NC_CACHE = _build_program()
    return _NC_CACHE


def _consts():
    import ml_dtypes

    bf16 = ml_dtypes.bfloat16
    p = np.arange(128)
    # rotate_half permutation-with-sign, transposed for matmul lhsT:
    # rot = SP @ raw, SP[f, f+32] = -1 (f%64 < 32), SP[f, f-32] = +1.
    sp = np.zeros((128, 128), dtype=np.float32)
    for b in (0, 64):
        for i in range(32):
            sp[b + i, b + i + 32] = -1.0
            sp[b + i + 32, b + i] = 1.0
    return {
        "ident": np.eye(128, dtype=np.float32),
        "spt": np.ascontiguousarray(sp.T).astype(bf16),
        "tri": (p[:, None] <= p[None, :]).astype(bf16),
        "ones8": np.ones((128, 8), dtype=bf16),
        "ones64": np.ones((1, 64), dtype=np.float32),
    }


def _in_maps(x, cos, sin, Wq, Wk, Wv, Wo):
    import ml_dtypes

    bf16 = ml_dtypes.bfloat16
    x = np.asarray(x, dtype=np.float32)
    cos = np.asarray(cos, dtype=np.float32)
    sin = np.asarray(sin, dtype=np.float32)
    Wq = np.asarray(Wq, dtype=np.float32)
    Wk = np.asarray(Wk, dtype=np.float32)
    Wv = np.asarray(Wv, dtype=np.float32)
    Wo = np.asarray(Wo, dtype=np.float32)

    cos2 = np.ascontiguousarray(np.tile(cos.T, (2, 1)))  # [128, T]
    sin2 = np.ascontiguousarray(np.tile(sin.T, (2, 1)))
    consts = _consts()

    in_maps = []
    for c in range(8):
        b, hh = c // 2, c % 2
        sl = slice(hh * FC, (hh + 1) * FC)
        in_maps.append(
            {
                "x": np.ascontiguousarray(x[b]),
                "wq": np.ascontiguousarray(Wq[:, sl]).astype(bf16),
                "wk": np.ascontiguousarray(Wk[:, sl]).astype(bf16),
                "wv": np.ascontiguousarray(Wv[:, sl]).astype(bf16),
                "wo": np.ascontiguousarray(Wo[sl, :]).astype(bf16),
                "cos2": cos2,
                "sin2": sin2,
                **consts,
            }
        )
    return in_maps


def _gather(results):
    outs = [results[c]["out"] for c in range(8)]
    return np.stack([outs[2 * b] + outs[2 * b + 1] for b in range(B)]).astype(
        np.float32
    )


def kernel(x, cos, sin, Wq, Wk, Wv, Wo):
    from concourse.bass_utils import run_bass_kernel_spmd

    in_maps = _in_maps(x, cos, sin, Wq, Wk, Wv, Wo)
    nc = _get_nc()
    res = run_bass_kernel_spmd(nc, in_maps, core_ids=list(range(8)))
    return _gather(res.results)
